# revision 3
# baseline (speedup 1.0000x reference)
"""Sparse 3D conv (gather -> per-offset matmul -> accumulate) on 8 TRN2 NeuronCores.

Strategy (data-parallel over output voxels, per sharding hint):
  - Shard the N=200000 output voxels across 8 cores (25000 each, padded to
    25088 = 49 tiles x 512 voxels).
  - Replicate the feature table (with appended zero rows) and the [27,32,64]
    kernel to every core's HBM.
  - Per (tile, chunk-of-128-voxels, k): one indirect DMA gathers 128 feature
    rows (masked entries redirected to zero rows) into SBUF in natural
    [voxel, ch] layout; PE transposes 128x128 blocks (4 k-slots x 32ch) to
    channel-major; f32r matmuls accumulate out^T[64, 512] over the 7 k-groups
    in PSUM; result DMA'd to a per-core out^T[64, 25088] buffer.
  - Host transposes/crops/concats per-core outputs to the full [200000, 64].
"""
import numpy as np
from contextlib import ExitStack

import concourse.bass as bass
import concourse.bacc as bacc
import concourse.mybir as mybir
import concourse.tile as tile
from concourse.bass_utils import run_bass_kernel_spmd
from concourse.masks import make_identity

N = 200000
K = 27
CIN = 32
COUT = 64
NCORES = 8
PERCORE = N // NCORES          # 25000
VTILE = 512
NTILES = (PERCORE + VTILE - 1) // VTILE   # 49
NPAD = NTILES * VTILE          # 25088
NCHUNK = VTILE // 128          # 4
KSLOTS = 28                    # 27 real + 1 zero pad slot (for 7 groups of 4)
NGRP = KSLOTS // 4             # 7
ZROWS = 64
NFEAT = N + ZROWS              # 200064

_NC_CACHE = None


def _build_kernel():
    nc = bacc.Bacc("TRN2", target_bir_lowering=False)
    feat = nc.dram_tensor("feat", [NFEAT, CIN], mybir.dt.float32, kind="ExternalInput")
    idx = nc.dram_tensor("idx", [NTILES, 128, NCHUNK * K], mybir.dt.int32,
                         kind="ExternalInput")
    wst = nc.dram_tensor("wst", [128, NGRP, COUT], mybir.dt.float32,
                         kind="ExternalInput")
    outT = nc.dram_tensor("outT", [COUT, NPAD], mybir.dt.float32,
                          kind="ExternalOutput")

    with tile.TileContext(nc) as tc, ExitStack() as ctx:
        const = ctx.enter_context(tc.tile_pool(name="const", bufs=1))
        sbi = ctx.enter_context(tc.tile_pool(name="sbi", bufs=3))
        sbg = ctx.enter_context(tc.tile_pool(name="sbg", bufs=3))
        sbt = ctx.enter_context(tc.tile_pool(name="sbt", bufs=2))
        sbo = ctx.enter_context(tc.tile_pool(name="sbo", bufs=2))
        tpsum = ctx.enter_context(tc.tile_pool(name="tpsum", bufs=4, space="PSUM"))
        opsum = ctx.enter_context(tc.tile_pool(name="opsum", bufs=2, space="PSUM"))

        ident = const.tile([128, 128], mybir.dt.float32, name="ident")
        make_identity(nc, ident[:])

        w_f32 = const.tile([128, NGRP, COUT], mybir.dt.float32, name="w_f32")
        nc.sync.dma_start(w_f32[:], wst[:])
        w_sb = const.tile([128, NGRP, COUT], mybir.dt.float32r, name="w_sb")
        nc.vector.tensor_copy(w_sb[:], w_f32[:])

        for t in range(NTILES):
            idx_t = sbi.tile([128, NCHUNK * K], mybir.dt.int32, name="idx_t",
                             tag="idx")
            nc.sync.dma_start(idx_t[:], idx[t])

            # natural-layout gather dest: [voxel-in-chunk, chunk, kslot, ch]
            g_nat = sbg.tile([128, NCHUNK, KSLOTS, CIN], mybir.dt.float32,
                             name="g_nat", tag="gnat")
            if t < 3:
                # Zero each of the 3 rotating pool buffers once; pad k-slot 27
                # is never written by gathers so it stays zero afterwards.
                nc.vector.memset(g_nat[:], 0.0)
            for chunk in range(NCHUNK):
                for k in range(K):
                    c = chunk * K + k
                    nc.gpsimd.indirect_dma_start(
                        out=g_nat[:, chunk, k, :],
                        out_offset=None,
                        in_=feat[:],
                        in_offset=bass.IndirectOffsetOnAxis(
                            ap=idx_t[:, c:c + 1], axis=0),
                    )

            gt = sbt.tile([128, NGRP, VTILE], mybir.dt.float32r, name="gt",
                          tag="gt")
            for chunk in range(NCHUNK):
                for g in range(NGRP):
                    pt = tpsum.tile([128, 128], mybir.dt.float32, name="pt",
                                    tag="tp")
                    nc.tensor.transpose(
                        pt[:],
                        g_nat[:, chunk, 4 * g:4 * g + 4, :],
                        ident[:],
                    )
                    dst = gt[:, g, chunk * 128:(chunk + 1) * 128]
                    if (chunk + g) % 2 == 0:
                        nc.vector.tensor_copy(out=dst, in_=pt[:])
                    else:
                        nc.scalar.copy(dst, pt[:])

            out_p = opsum.tile([COUT, VTILE], mybir.dt.float32, name="out_p",
                               tag="op")
            for g in range(NGRP):
                nc.tensor.matmul(
                    out_p[:],
                    lhsT=w_sb[:, g, :],
                    rhs=gt[:, g, :],
                    start=(g == 0),
                    stop=(g == NGRP - 1),
                )
            out_sb = sbo.tile([COUT, VTILE], mybir.dt.float32, name="out_sb",
                              tag="ob")
            nc.vector.tensor_copy(out_sb[:], out_p[:])
            nc.sync.dma_start(outT[:, t * VTILE:(t + 1) * VTILE], out_sb[:])

    nc.compile()
    return nc


def _get_nc():
    global _NC_CACHE
    if _NC_CACHE is None:
        _NC_CACHE = _build_kernel()
    return _NC_CACHE


def _prep_host(features, neighbor_map, neighbor_mask, kernel):
    feat_pad = np.zeros((NFEAT, CIN), dtype=np.float32)
    feat_pad[:N] = np.asarray(features, dtype=np.float32)

    nm = np.asarray(neighbor_map, dtype=np.int64)      # [27, N]
    mk = np.asarray(neighbor_mask, dtype=bool)          # [27, N]

    # weight stack: group g rows 32j..32j+31 = kernel[4g+j]; pad slot -> 0
    w = np.asarray(kernel, dtype=np.float32)            # [27, 32, 64]
    wstk = np.zeros((NGRP, 4, CIN, COUT), dtype=np.float32)
    for g in range(NGRP):
        for j in range(4):
            kk = 4 * g + j
            if kk < K:
                wstk[g, j] = w[kk]
    wst = wstk.transpose(1, 2, 0, 3).reshape(128, NGRP, COUT).copy()

    idx_all = []
    for c in range(NCORES):
        vloc = np.arange(NPAD)
        vglob = np.minimum(c * PERCORE + vloc, N - 1)
        valid_v = vloc < PERCORE                        # [NPAD]
        nmv = nm[:, vglob]                              # [27, NPAD]
        mskv = mk[:, vglob] & valid_v[None, :]
        kk = np.arange(K)[:, None]
        zrow = N + (vglob[None, :] * 7 + kk * 13) % ZROWS
        vals = np.where(mskv, nmv, zrow).astype(np.int32)   # [27, NPAD]
        # [27, NPAD] -> [49, 128, 4*27] with col = chunk*27 + k
        v4 = vals.reshape(K, NTILES, NCHUNK, 128)
        v4 = v4.transpose(1, 3, 2, 0)                   # [49, 128, chunk, k]
        idx_all.append(np.ascontiguousarray(v4.reshape(NTILES, 128, NCHUNK * K)))
    return feat_pad, wst, idx_all


def kernel(features, neighbor_map, neighbor_mask, kernel):
    feat_pad, wst, idx_all = _prep_host(features, neighbor_map, neighbor_mask,
                                        kernel)
    nc = _get_nc()
    in_maps = [
        {"feat": feat_pad, "idx": idx_all[c], "wst": wst}
        for c in range(NCORES)
    ]
    res = run_bass_kernel_spmd(nc, in_maps, core_ids=list(range(NCORES)))
    outs = []
    for c in range(NCORES):
        oT = res.results[c]["outT"]                     # [64, NPAD]
        outs.append(oT.T[:PERCORE])                     # [25000, 64]
    return np.concatenate(outs, axis=0).astype(np.float32)


# revision 4
# speedup vs baseline: 1186.2978x; 1186.2978x over previous
"""Sparse 3D conv (gather -> per-offset matmul -> accumulate) on 8 TRN2 NeuronCores.

Strategy (data-parallel over output voxels, per sharding hint):
  - Shard the N=200000 output voxels across 8 cores (25000 each, padded to
    25088 = 49 tiles x 512 voxels).
  - Replicate the feature table (with appended zero rows) and the [27,32,64]
    kernel to every core's HBM.
  - Per (tile, chunk-of-128-voxels, k): one indirect DMA gathers 128 feature
    rows (masked entries redirected to zero rows) into SBUF in natural
    [voxel, ch] layout; PE transposes 128x128 blocks (4 k-slots x 32ch) to
    channel-major; f32r matmuls accumulate out^T[64, 512] over the 7 k-groups
    in PSUM; result DMA'd to a per-core out^T[64, 25088] buffer.
  - Host transposes/crops/concats per-core outputs to the full [200000, 64].
"""
import numpy as np
from contextlib import ExitStack

import concourse.bass as bass
import concourse.bacc as bacc
import concourse.mybir as mybir
import concourse.tile as tile
from concourse.bass_utils import run_bass_kernel_spmd
from concourse.masks import make_identity

N = 200000
K = 27
CIN = 32
COUT = 64
NCORES = 8
PERCORE = N // NCORES          # 25000
VTILE = 512
NTILES = (PERCORE + VTILE - 1) // VTILE   # 49
NPAD = NTILES * VTILE          # 25088
NCHUNK = VTILE // 128          # 4
KSLOTS = 28                    # 27 real + 1 zero pad slot (for 7 groups of 4)
NGRP = KSLOTS // 4             # 7
ZROWS = 64
NFEAT = N + ZROWS              # 200064

_NC_CACHE = None


def _build_kernel():
    nc = bacc.Bacc("TRN2", target_bir_lowering=False)
    feat = nc.dram_tensor("feat", [NFEAT, CIN], mybir.dt.float32, kind="ExternalInput")
    idx = nc.dram_tensor("idx", [NTILES, 128, NCHUNK * K], mybir.dt.int32,
                         kind="ExternalInput")
    wst = nc.dram_tensor("wst", [128, NGRP, COUT], mybir.dt.float32,
                         kind="ExternalInput")
    outT = nc.dram_tensor("outT", [COUT, NPAD], mybir.dt.float32,
                          kind="ExternalOutput")

    with tile.TileContext(nc) as tc, ExitStack() as ctx:
        const = ctx.enter_context(tc.tile_pool(name="const", bufs=1))
        sbg = ctx.enter_context(tc.tile_pool(name="sbg", bufs=4))
        sbt = ctx.enter_context(tc.tile_pool(name="sbt", bufs=2))
        sbo = ctx.enter_context(tc.tile_pool(name="sbo", bufs=2))
        tpsum = ctx.enter_context(tc.tile_pool(name="tpsum", bufs=6, space="PSUM"))
        opsum = ctx.enter_context(tc.tile_pool(name="opsum", bufs=2, space="PSUM"))

        ident = const.tile([128, 128], mybir.dt.float32, name="ident")
        make_identity(nc, ident[:])

        idx_sb = const.tile([128, NTILES, NCHUNK * K], mybir.dt.int32,
                            name="idx_sb")
        nc.sync.dma_start(idx_sb[:], idx[:].rearrange("t p c -> p t c"))

        w_f32 = const.tile([128, NGRP, COUT], mybir.dt.float32, name="w_f32")
        nc.sync.dma_start(w_f32[:], wst[:])
        w_sb = const.tile([128, NGRP, COUT], mybir.dt.float32r, name="w_sb")
        nc.vector.tensor_copy(w_sb[:], w_f32[:])

        for t in range(NTILES):
            # natural-layout gather dest: [voxel-in-chunk, chunk, kslot, ch]
            g_nat = sbg.tile([128, NCHUNK, KSLOTS, CIN], mybir.dt.float32,
                             name="g_nat", tag="gnat")
            if t < 4:
                # Zero each of the 3 rotating pool buffers once; pad k-slot 27
                # is never written by gathers so it stays zero afterwards.
                nc.vector.memset(g_nat[:], 0.0)
            for chunk in range(NCHUNK):
                for k in range(K):
                    c = chunk * K + k
                    nc.gpsimd.indirect_dma_start(
                        out=g_nat[:, chunk, k, :],
                        out_offset=None,
                        in_=feat[:],
                        in_offset=bass.IndirectOffsetOnAxis(
                            ap=idx_sb[:, t, c:c + 1], axis=0),
                    )

            gt = sbt.tile([128, NGRP, VTILE], mybir.dt.float32r, name="gt",
                          tag="gt")
            for chunk in range(NCHUNK):
                for g in range(NGRP):
                    pt = tpsum.tile([128, 128], mybir.dt.float32, name="pt",
                                    tag="tp")
                    nc.tensor.transpose(
                        pt[:],
                        g_nat[:, chunk, 4 * g:4 * g + 4, :],
                        ident[:],
                    )
                    dst = gt[:, g, chunk * 128:(chunk + 1) * 128]
                    if (chunk + g) % 2 == 0:
                        nc.vector.tensor_copy(out=dst, in_=pt[:])
                    else:
                        nc.scalar.copy(dst, pt[:])

            out_p = opsum.tile([COUT, VTILE], mybir.dt.float32, name="out_p",
                               tag="op")
            for g in range(NGRP):
                nc.tensor.matmul(
                    out_p[:],
                    lhsT=w_sb[:, g, :],
                    rhs=gt[:, g, :],
                    start=(g == 0),
                    stop=(g == NGRP - 1),
                )
            out_sb = sbo.tile([COUT, VTILE], mybir.dt.float32, name="out_sb",
                              tag="ob")
            nc.vector.tensor_copy(out_sb[:], out_p[:])
            nc.sync.dma_start(outT[:, t * VTILE:(t + 1) * VTILE], out_sb[:])

    nc.compile()
    return nc


def _get_nc():
    global _NC_CACHE
    if _NC_CACHE is None:
        _NC_CACHE = _build_kernel()
    return _NC_CACHE


def _prep_host(features, neighbor_map, neighbor_mask, kernel):
    feat_pad = np.zeros((NFEAT, CIN), dtype=np.float32)
    feat_pad[:N] = np.asarray(features, dtype=np.float32)

    nm = np.asarray(neighbor_map, dtype=np.int64)      # [27, N]
    mk = np.asarray(neighbor_mask, dtype=bool)          # [27, N]

    # weight stack: group g rows 32j..32j+31 = kernel[4g+j]; pad slot -> 0
    w = np.asarray(kernel, dtype=np.float32)            # [27, 32, 64]
    wstk = np.zeros((NGRP, 4, CIN, COUT), dtype=np.float32)
    for g in range(NGRP):
        for j in range(4):
            kk = 4 * g + j
            if kk < K:
                wstk[g, j] = w[kk]
    wst = wstk.transpose(1, 2, 0, 3).reshape(128, NGRP, COUT).copy()

    idx_all = []
    for c in range(NCORES):
        vloc = np.arange(NPAD)
        vglob = np.minimum(c * PERCORE + vloc, N - 1)
        valid_v = vloc < PERCORE                        # [NPAD]
        nmv = nm[:, vglob]                              # [27, NPAD]
        mskv = mk[:, vglob] & valid_v[None, :]
        kk = np.arange(K)[:, None]
        zrow = N + (vglob[None, :] * 7 + kk * 13) % ZROWS
        vals = np.where(mskv, nmv, zrow).astype(np.int32)   # [27, NPAD]
        # [27, NPAD] -> [49, 128, 4*27] with col = chunk*27 + k
        v4 = vals.reshape(K, NTILES, NCHUNK, 128)
        v4 = v4.transpose(1, 3, 2, 0)                   # [49, 128, chunk, k]
        idx_all.append(np.ascontiguousarray(v4.reshape(NTILES, 128, NCHUNK * K)))
    return feat_pad, wst, idx_all


def kernel(features, neighbor_map, neighbor_mask, kernel):
    feat_pad, wst, idx_all = _prep_host(features, neighbor_map, neighbor_mask,
                                        kernel)
    nc = _get_nc()
    in_maps = [
        {"feat": feat_pad, "idx": idx_all[c], "wst": wst}
        for c in range(NCORES)
    ]
    res = run_bass_kernel_spmd(nc, in_maps, core_ids=list(range(NCORES)))
    outs = []
    for c in range(NCORES):
        oT = res.results[c]["outT"]                     # [64, NPAD]
        outs.append(oT.T[:PERCORE])                     # [25000, 64]
    return np.concatenate(outs, axis=0).astype(np.float32)
